# revision 18
# baseline (speedup 1.0000x reference)
"""Distributed GPT-2 attention block for 8 TRN2 NeuronCores — collective-free.

Sharding: core c handles batch b = c//4 and head-group g = c%4 (heads
4g..4g+3, as two row-packed pairs). Each core computes QKV for its 2048
tokens x 768 qkv-features, attention for its 4 heads, and a PARTIAL c_proj
over its 256 feature rows; the host sums the 4 partial outputs per batch
(the tensor-parallel all-reduce done at unshard time).

Layout: hidden_states pre-transposed [NX, S] bf16; q,k kept in [d, t]
layout so scores come out as [keys, queries]; v natural [t, d] with an
appended ones-column so the PV matmul also yields softmax denominators.
Score matmuls for the two heads of a pair are row-packed (head 0 in PE rows
0-63, head 1 in rows 64-127) into one 2-bank PSUM tile, so they run
concurrently and a single ACT pass exps both heads.

Causal masking: block skipping + width-shrunk diagonal tiles + a mask
MATMUL accumulated into the score PSUM group (lhsT = strict-upper -2e5
triangle, rhs = identity adds -2e5 where key>query; exp then gives exact
zeros). This keeps GpSimd entirely out of the score->exp->PV chain.

Schedule: the ACT exp stream is the attention pacer, so everything else is
woven around it. QKV matmul groups chase the DMA arrivals (one batched DMA
per 512-token chunk on the otherwise-idle GpSimd issue queue; small
latency-critical DMAs ride the Sync queue). Attention emission is
software-pipelined (scores of tile t+1 issue before PV of tile t) and each
tile consumes one small "filler" chunk - remaining QKV groups during
pair-0 blocks, partial-c_proj column tiles during pair-1 blocks - so the
PE never idles while ACT exps and never bursts while ACT starves.
c_proj output tiles are staged in SBUF and shipped with one 1MB DMA per
512-token slot. All matmuls accumulate in f32 PSUM; compute dtype bf16.
"""

import numpy as np
import ml_dtypes
from contextlib import ExitStack

import concourse.bass as bass
import concourse.bacc as bacc
import concourse.mybir as mybir
import concourse.tile as tile
from concourse.bass_utils import run_bass_kernel_spmd

B, S, NX = 2, 2048, 1024
H, D = 16, 64
HPG = 4              # heads per core (one group)
GF = HPG * D         # 256 v-features per group
NCORES = 8
KT = NX // 128       # 8 contraction tiles of the 1024 feature axis
NTCH = S // 512      # 4 query/token chunks of 512

F32 = mybir.dt.float32
BF16 = mybir.dt.bfloat16
IDENT = mybir.ActivationFunctionType.Identity
MASKV = -200000.0    # x0.125 exp scale -> exp(-25000) == 0


def build(zero_attn_bias: bool) -> bass.Bass:
    nc = bacc.Bacc(None)

    hst = nc.declare_dram_parameter("hst", [NX, S], BF16, isOutput=False)
    # cols: [q-pair0(128) | k-pair0(128) | q-pair1(128) | k-pair1(128) | v(256)]
    wqkv = nc.declare_dram_parameter("wqkv", [NX, 768], BF16, isOutput=False)
    bqkv = nc.declare_dram_parameter("bqkv", [768, 1], F32, isOutput=False)
    wproj = nc.declare_dram_parameter("wproj", [GF, NX], BF16, isOutput=False)
    out_ext = nc.declare_dram_parameter("out", [NX, S], BF16, isOutput=True)

    with tile.TileContext(nc) as tc, ExitStack() as ctx:
        pool1 = ctx.enter_context(tc.tile_pool(name="persist", bufs=1))
        small = ctx.enter_context(tc.tile_pool(name="small", bufs=2))
        ppool = ctx.enter_context(tc.tile_pool(name="ppool", bufs=3))
        ptpool = ctx.enter_context(tc.tile_pool(name="ptpool", bufs=4))
        opool = ctx.enter_context(tc.tile_pool(name="opool", bufs=2))
        # PSUM budget (8 banks): st 2x2 + av 1x2 + pq 2x1
        psum_s = ctx.enter_context(tc.tile_pool(name="psum_s", bufs=2, space="PSUM"))
        psum_av = ctx.enter_context(tc.tile_pool(name="psum_av", bufs=1, space="PSUM"))
        psum_pq = ctx.enter_context(tc.tile_pool(name="psum_pq", bufs=2, space="PSUM"))

        # ---- constant masks / helpers ---------------------------------------
        # mtri[p, y] = MASKV iff y > p: as score-matmul lhsT with identity rhs
        # it adds MASKV exactly where key > query on a diagonal tile.
        mtri = pool1.tile([128, 128], BF16)
        nc.gpsimd.memset(mtri[:], MASKV)
        nc.gpsimd.affine_select(
            out=mtri[:], in_=mtri[:],
            compare_op=mybir.AluOpType.is_gt, fill=0.0,
            base=0, pattern=[[1, 128]], channel_multiplier=-1)
        eye = pool1.tile([128, 128], BF16)
        nc.gpsimd.memset(eye[:], 1.0)
        nc.gpsimd.affine_select(
            out=eye[:], in_=eye[:],
            compare_op=mybir.AluOpType.is_equal, fill=0.0,
            base=0, pattern=[[1, 128]], channel_multiplier=-1)
        ones1 = pool1.tile([1, D], BF16)
        nc.gpsimd.memset(ones1[:], 1.0)

        # v natural [token, feat] + ones column: [128, tt, habs, 65]
        v_sb = pool1.tile([128, S // 128, HPG, D + 1], BF16)
        nc.gpsimd.memset(v_sb[:, :, :, D:D + 1], 1.0)

        # ---- loads: issued from the Sync queue (free at preamble end), in
        # first-needed order; the first hst chunk is split so the first qk
        # matmul chain can start chasing the kt tiles as they land.
        wqkv_bf = pool1.tile([128, KT, 768], BF16)
        hst_bf = pool1.tile([128, KT, S], BF16)
        nc.sync.dma_start(
            wqkv_bf[:, :, 0:256],
            wqkv[:, 0:256].rearrange("(kt p) c -> p kt c", p=128))
        nc.sync.dma_start(
            hst_bf[:, 0:4, 0:512],
            hst[0:512, 0:512].rearrange("(kt p) t -> p kt t", p=128))
        nc.sync.dma_start(
            hst_bf[:, 4:8, 0:512],
            hst[512:1024, 0:512].rearrange("(kt p) t -> p kt t", p=128))
        nc.sync.dma_start(
            wqkv_bf[:, :, 512:768],
            wqkv[:, 512:768].rearrange("(kt p) c -> p kt c", p=128))
        for tch in range(1, NTCH):
            tsl = slice(tch * 512, (tch + 1) * 512)
            nc.sync.dma_start(
                hst_bf[:, :, tsl],
                hst[:, tsl].rearrange("(kt p) t -> p kt t", p=128))
        nc.sync.dma_start(
            wqkv_bf[:, :, 256:512],
            wqkv[:, 256:512].rearrange("(kt p) c -> p kt c", p=128))
        wproj_bf = pool1.tile([128, 2, NX], BF16)
        nc.sync.dma_start(
            wproj_bf[:], wproj[:, :].rearrange("(kt p) n -> p kt n", p=128))

        # biases (q/k: feature-per-partition in the [d, t] layout)
        bqk_t = pool1.tile([128, 2, 2, 1], F32)   # [*, pair, ft, 1]
        bv_t = pool1.tile([64, HPG, 1], F32)
        if not zero_attn_bias:
            for p in range(2):
                for ft in range(2):
                    o = 256 * p + 128 * ft
                    nc.sync.dma_start(bqk_t[:, p, ft, :], bqkv[o:o + 128, :])
            for h in range(HPG):
                nc.sync.dma_start(
                    bv_t[:, h, :], bqkv[512 + h * D:512 + (h + 1) * D, :])

        # ---- QKV projection --------------------------------------------------
        # q,k transposed: qk_sb[:, pair, ft, t]; ft 0 = q, 1 = k
        qk_sb = pool1.tile([128, 2, 2, S], BF16)

        def qk_half(p, ft, tch, half, box, act=False):
            # half of a 512-token q/k chunk: filler-sized (~0.9us) emission
            if half == 0:
                box["ps"] = psum_pq.tile([128, 512], F32, tag="pq",
                                         name=f"pqk{p}{ft}{tch}")
            ps = box["ps"]
            for kt in range(4 * half, 4 * half + 4):
                nc.tensor.matmul(
                    ps[:],
                    lhsT=wqkv_bf[:, kt, 256 * p + 128 * ft:256 * p + 128 * ft + 128],
                    rhs=hst_bf[:, kt, tch * 512:(tch + 1) * 512],
                    start=(kt == 0), stop=(kt == KT - 1),
                )
            if half == 0:
                return
            dst = qk_sb[:, p, ft, tch * 512:(tch + 1) * 512]
            if not zero_attn_bias:
                nc.scalar.activation(dst, ps[:], IDENT, bias=bqk_t[:, p, ft, :])
            elif act:
                nc.scalar.copy(dst, ps[:])
            else:
                nc.vector.tensor_copy(dst, ps[:])

        def qk_group(p, ft, tch, act=False):
            box = {}
            qk_half(p, ft, tch, 0, box, act)
            qk_half(p, ft, tch, 1, box, act)

        def qk_chunks(p, ft, tch, act=False):
            box = {}
            return [(lambda: qk_half(p, ft, tch, 0, box, act)),
                    (lambda: qk_half(p, ft, tch, 1, box, act))]

        def v_group(p, tt):
            # one 128-token tile of v for pair p (2 heads, N=128)
            ps = psum_pq.tile([128, 128], F32, tag="pq")
            for kt in range(KT):
                nc.tensor.matmul(
                    ps[:],
                    lhsT=hst_bf[:, kt, tt * 128:(tt + 1) * 128],
                    rhs=wqkv_bf[:, kt, 512 + 128 * p:640 + 128 * p],
                    start=(kt == 0), stop=(kt == KT - 1),
                )
            nc.vector.tensor_copy(
                v_sb[:, tt, 2 * p:2 * p + 2, 0:D],
                ps[:].rearrange("p (h d) -> p h d", h=2))

        # ---- attention -------------------------------------------------------
        # c_proj rhs layout: [feat(2 heads stacked on partitions), pair, slot, 512]
        atall = pool1.tile([128, 2, NTCH, 512], BF16)

        pending = []

        def attn_block(p, tb, fill=(), last=False):
            """One 512-query block for head pair p, software-pipelined at
            depth 2: the scores of tiles t+1,t+2 issue before the PV of tile
            t (so the PV's wait on the previous block's PSUM snapshot never
            head-of-line-blocks the score/exp stream), and one filler chunk
            is woven in per tile so the PE stays fed while ACT exps."""
            fill = list(fill)
            q0 = tb * 512
            av = psum_av.tile([D + 1, 2, 512], F32, tag="av")
            ntj = 4 * (tb + 1)
            geom = []
            for t in range(ntj):
                u = t - 4 * tb              # >=0 only inside diagonal quad
                w = 512 if u < 0 else 512 - 128 * u
                geom.append((t, u, 512 - w))

            pts = [None] * ntj

            def scores(t, u, c0):
                st = psum_s.tile([128, 2, 512], F32, tag="st")
                for h in range(2):          # row-packed pair: concurrent MMs
                    nc.tensor.matmul(
                        st[:, h, c0:512],
                        lhsT=qk_sb[64 * h:64 * h + 64, p, 1,
                                   t * 128:(t + 1) * 128],
                        rhs=qk_sb[64 * h:64 * h + 64, p, 0, q0 + c0:q0 + 512],
                        start=True, stop=(u < 0),
                    )
                if u >= 0:
                    # add -2e5 where key>query, joins the PSUM accum group
                    for h in range(2):
                        nc.tensor.matmul(
                            st[:, h, c0:c0 + 128],
                            lhsT=mtri[:], rhs=eye[:],
                            start=False, stop=True,
                        )
                pt = ptpool.tile([128, 2, 512], BF16, tag="pt")
                nc.scalar.activation(
                    pt[:, :, c0:512], st[:, :, c0:512],
                    mybir.ActivationFunctionType.Exp, scale=0.125)
                pts[t] = pt

            def pv(i):
                t, _, c0 = geom[i]          # noqa: shadows loop var safely
                for h in range(2):
                    nc.tensor.matmul(
                        av[:, h, c0:512],
                        lhsT=v_sb[:, t, 2 * p + h, :],
                        rhs=pts[t][:, h, c0:512],
                        start=(t == 0), stop=(t == ntj - 1),
                    )
                pts[t] = None

            scores(*geom[0])
            for i in range(1, ntj):
                scores(*geom[i])
                if i == 1 and pending:
                    pending.pop(0)()
                elif fill:
                    fill.pop(0)()
                pv(i - 1)
            pv(ntj - 1)
            for f in fill:                  # leftover fillers
                f()

            # snapshot numerators + denominators out of PSUM at block end so
            # the av slot frees immediately (on ACT for the final blocks,
            # whose snapshot lands after the exp stream has ended)
            avs = ppool.tile([D + 1, 2, 512], BF16, tag="avs")
            if last:
                nc.scalar.copy(avs[:], av[:])
            else:
                nc.vector.tensor_copy(avs[:], av[:])
            # reciprocal cost scales with free-size per partition: DMA the
            # denominator row across 128 partitions, recip there, DMA back
            dent = small.tile([128, 2 * 512 // 128], BF16, tag="dent")
            nc.sync.dma_start(dent[:], avs[D:D + 1, :, :])
            recp = small.tile([128, 2 * 512 // 128], BF16, tag="recp")
            with nc.allow_low_precision("softmax recip bf16 is fine"):
                nc.vector.reciprocal(recp[:], dent[:])
            r2 = small.tile([1, 2, 512], BF16, tag="r2")
            nc.sync.dma_start(r2[:], recp[:])

            def make_epilogue(avs=avs, r2=r2, p=p, tb=tb):
                def epi():
                    for h in range(2):
                        rb = psum_pq.tile([D, 512], F32, tag="pq",
                                          name=f"rb{p}{tb}{h}")
                        nc.tensor.matmul(rb[:], lhsT=ones1[:],
                                         rhs=r2[:, h, :],
                                         start=True, stop=True)
                        at = ppool.tile([D, 512], BF16, tag="at")
                        if zero_attn_bias:
                            nc.vector.tensor_mul(at[:], avs[0:D, h, :], rb[:])
                        else:
                            at0 = ppool.tile([D, 512], BF16, tag="at0")
                            nc.vector.tensor_mul(at0[:], avs[0:D, h, :], rb[:])
                            nc.scalar.activation(
                                at[:], at0[:], IDENT,
                                bias=bv_t[:, 2 * p + h, :])
                        nc.gpsimd.dma_start(
                            atall[64 * h:64 * h + 64, p, tb, :], at[:])
                return epi
            pending.append(make_epilogue())

        # partial c_proj, one 128-row column tile at a time (filler-sized);
        # results stage in SBUF and ship as one DMA per slot
        ot_slots = {}

        def proj_ntile(s, n, act=False, half_dma=False):
            def go():
                if n == 0:
                    ot_slots[s] = opool.tile([128, KT, 512], BF16, tag="ot",
                                             name=f"ot{s}")
                ps = psum_pq.tile([128, 512], F32, tag="pq")
                for kt in range(2):
                    nc.tensor.matmul(
                        ps[:],
                        lhsT=wproj_bf[:, kt, n * 128:(n + 1) * 128],
                        rhs=atall[:, kt, s, :],
                        start=(kt == 0), stop=(kt == 1),
                    )
                ot = ot_slots[s]
                if act:
                    nc.scalar.copy(ot[:, n, :], ps[:])
                else:
                    nc.vector.tensor_copy(ot[:, n, :], ps[:])
                cuts = {3: (0, 4), 5: (4, 6), 7: (6, 8)} if half_dma \
                    else {7: (0, 8)}
                if n in cuts:
                    lo, hi = cuts[n]
                    nc.gpsimd.dma_start(
                        out_ext[lo * 128:hi * 128,
                                s * 512:(s + 1) * 512].rearrange(
                            "(n p) t -> p n t", p=128),
                        ot[:, lo:hi, :])
            return go

        # ---- schedule --------------------------------------------------------
        F = lambda fn, *a, **k: (lambda: fn(*a, **k))

        qk_group(0, 0, 0, act=True)
        qk_group(0, 1, 0, act=True)
        v_group(0, 0)
        attn_block(0, 0, [F(v_group, 0, 1), F(v_group, 0, 2), F(v_group, 0, 3)])
        qk_group(0, 0, 1, act=True)
        qk_group(0, 1, 1, act=True)
        attn_block(0, 1, [F(v_group, 0, 4), F(v_group, 0, 5)] +
                         qk_chunks(1, 1, 0) +
                         [F(v_group, 0, 6), F(v_group, 0, 7)])
        qk_group(0, 0, 2)
        qk_group(0, 1, 2)
        attn_block(0, 2, [F(v_group, 0, tt) for tt in (8, 9, 10, 11)] +
                         qk_chunks(1, 1, 1) +
                         [F(v_group, 1, tt) for tt in (0, 1, 2, 3)])
        qk_group(0, 0, 3)
        qk_group(0, 1, 3)
        attn_block(0, 3, [F(v_group, 0, tt) for tt in (12, 13, 14, 15)] +
                         qk_chunks(1, 1, 2) + qk_chunks(1, 1, 3) +
                         qk_chunks(1, 0, 3) +
                         [F(v_group, 1, tt) for tt in (4, 5, 6, 7)])
        attn_block(1, 3, [F(v_group, 1, tt) for tt in range(8, 16)] +
                         qk_chunks(1, 0, 2) + qk_chunks(1, 0, 1))
        attn_block(1, 2, qk_chunks(1, 0, 0) +
                         [proj_ntile(3, n) for n in range(8)])
        attn_block(1, 1, [proj_ntile(2, n) for n in range(6)])
        proj_ntile(2, 6)()
        proj_ntile(2, 7)()
        attn_block(1, 0, [proj_ntile(1, 0), proj_ntile(1, 1)], last=True)
        while pending:
            pending.pop(0)()
        # tail: remaining slots with both evac engines alternating so the
        # PE stays dense (and warm) to the end; output ships in pieces
        tail = []
        for n in range(2, 8):
            tail.append(proj_ntile(1, n, act=(n % 2 == 0)))
        for n in range(8):
            tail.append(proj_ntile(0, n, act=(n % 2 == 1), half_dma=True))
        for f in tail:
            f()

    nc.finalize()
    return nc


_CACHE = {}


def _get_nc(zero_attn_bias):
    if zero_attn_bias not in _CACHE:
        _CACHE[zero_attn_bias] = build(zero_attn_bias)
    return _CACHE[zero_attn_bias]


def kernel(hidden_states, c_attn_w, c_attn_b, c_proj_w, c_proj_b, **extra):
    hidden_states = np.asarray(hidden_states, np.float32)
    c_attn_w = np.asarray(c_attn_w, np.float32)
    c_attn_b = np.asarray(c_attn_b, np.float32)
    c_proj_w = np.asarray(c_proj_w, np.float32)
    c_proj_b = np.asarray(c_proj_b, np.float32)

    zero_attn_bias = not np.any(c_attn_b)
    nc = _get_nc(zero_attn_bias)

    bf = ml_dtypes.bfloat16
    hsT = [np.ascontiguousarray(hidden_states[b].T).astype(bf)
           for b in range(B)]

    in_maps = []
    for c in range(NCORES):
        b, g = divmod(c, 4)
        q0 = 256 * g
        cols = np.r_[q0:q0 + 128,                    # q pair 0
                     NX + q0:NX + q0 + 128,          # k pair 0
                     q0 + 128:q0 + 256,              # q pair 1
                     NX + q0 + 128:NX + q0 + 256,    # k pair 1
                     2 * NX + q0:2 * NX + q0 + 256]  # v (4 heads)
        in_maps.append({
            "hst": hsT[b],
            "wqkv": np.ascontiguousarray(c_attn_w[:, cols]).astype(bf),
            "bqkv": np.ascontiguousarray(c_attn_b[cols].reshape(768, 1)),
            "wproj": np.ascontiguousarray(
                c_proj_w[q0:q0 + 256, :]).astype(bf),
        })

    res = run_bass_kernel_spmd(nc, in_maps, core_ids=list(range(NCORES)))
    out = np.empty((B, S, NX), np.float32)
    for b in range(B):
        acc = np.zeros((NX, S), np.float32)
        for g in range(4):
            acc += np.asarray(res.results[4 * b + g]["out"]).astype(np.float32)
        out[b] = acc.T + c_proj_b[None, :]
    return out


if __name__ == "__main__":
    rng = np.random.default_rng(0)
    hs = rng.standard_normal((B, S, NX), dtype=np.float32)
    wa = (rng.standard_normal((NX, 3 * NX), dtype=np.float32) * 0.02)
    wp = (rng.standard_normal((NX, NX), dtype=np.float32) * 0.02)
    o = kernel(hidden_states=hs, c_attn_w=wa, c_attn_b=np.zeros(3 * NX, np.float32),
               c_proj_w=wp, c_proj_b=np.zeros(NX, np.float32))
    print(o.shape, o.dtype)
